# revision 12
# baseline (speedup 1.0000x reference)
"""Trainium2 Bass kernel for nn_MultiHeadSelfAttention_62646392979761.

Math (per the buggy-einsum reference): per position s, heads attend to heads:
  Q,K,V = x@W{q,k,v}.T + b  (N,S,H,D);  scores[s] = Q[s]K[s]^T/8 (16x16);
  A = softmax_j;  AV[s] = A[s]V[s];  out2 = scrambled reshape (16-position
  groups of one head per output row);  final = out2@Wo.T + bo.

Sharding: 8 cores x 2048 rows of the flattened (16384, 1024) x. Attention is
position-local; the scramble groups 16 consecutive positions, which never
cross a 2048-row shard. Zero cross-core communication.

Per-core device pipeline (16 tiles x 128 positions), all layouts validated
against the reference in a numpy simulator:
  1. QKV projections: PE matmuls, stationary xT e-chunks, moving fused
     [Wq/8|Wk|Wv]^T bf16; DVE bias-add evac -> QN/KN/VN bf16.
  2. xbar DMA transposes -> QT/KT chunks [(i2,d), slot].
  3. QBLK (masked block-diag pairs) + KBLK via 4 fused strided DVE copies
     each; structural zeros memset once.
  4. Scores: 64 pair-packed matmuls (k=128, m=32 col-rotated, n=16) ->
     SCO psum [(u,p,i), (j,gg)].
  5. ACT exp -> E bf16; DVE j-reduce -> Z; reciprocal -> Zr.
  6. E -> DRAM bounce -> ABLK [(p,j), (p,i,u,gg)] (masked, 32B-run DMA);
     VN -> DRAM bounce -> VTHP [(p,j), (g,d)] (128B-run DMA).
  7. AV: 64 pair matmuls (k=32, m=32 col-rotated, n=64) -> ANAT psum
     [(u,p,i), (gg,d)]; evac * Zr (per-gg tensor_scalar) -> bf16.
  8. xbar ANAT slices -> AVB2 [(b,d), (t,g',u,p,i)] accumulated all tiles.
  9. Final projection: host-permuted WoT chunks stationary, AVB2 strided
     rhs, 8-chunk psum accumulation, + bo -> outT (1024, 2048) f32.
Host: pre-permutes x rows (even/odd within tile), pre-transposes weights,
post-scatters finalT columns to (n, s_out) rows.
"""

import math
import numpy as np
import ml_dtypes

TILES = 16
ROWS = 2048
NB, SB, EB, HB, DB = 4, 4096, 1024, 16, 64

_CACHE = {}


def _split_waits_json(bir_bytes):
    """This env's walrus accepts only ONE embedded sync-wait per TPB
    instruction (NEURON_ISA_TPB_EVENTS has a single wait slot) but Tile emits
    several. Split excess on_wait entries onto standalone EventSemaphore
    instructions inserted just before, on the same engine — semantically
    identical on in-order engine queues."""
    import json
    d = json.loads(bir_bytes)
    for fn in d.get('functions', []):
        for bb in (fn.get('basic_blocks') or fn.get('blocks') or []):
            out = []
            for inst in bb.get('instructions', []):
                si = inst.get('sync_info')
                w = (si or {}).get('on_wait') or []
                if len(w) > 1:
                    for k, extra in enumerate(w[:-1]):
                        out.append({
                            'debug': inst.get('debug', 0),
                            'engine': inst['engine'],
                            'ins': [], 'outs': [],
                            'name': f"{inst['name']}-sw{k}",
                            'opcode': 'EventSemaphore',
                            'sync_info': {'on_wait': [extra], 'on_update': []},
                        })
                    si['on_wait'] = [w[-1]]
                out.append(inst)
            bb['instructions'] = out
    return json.dumps(d).encode()


def _install_birpatch():
    import concourse.bass_utils as bu
    import concourse.bass2jax as b2j
    if getattr(bu.compile_bir_kernel, '_waitsplit', False):
        return
    orig = bu.compile_bir_kernel

    def patched(bir_json, tmpdir, neff_name="file.neff"):
        return orig(_split_waits_json(bir_json), tmpdir, neff_name)

    patched._waitsplit = True
    bu.compile_bir_kernel = patched
    b2j.compile_bir_kernel = patched


def _build_bass():
    import concourse.bass as bass
    import concourse.tile as tile
    from concourse import mybir

    bf16 = mybir.dt.bfloat16
    f32 = mybir.dt.float32
    AF = mybir.ActivationFunctionType
    ALU = mybir.AluOpType
    AX = mybir.AxisListType

    nc = bass.Bass(trn_type="TRN2")
    xt_d = nc.declare_dram_parameter("xt", [1024, ROWS], bf16, isOutput=False)
    wqkv_d = nc.declare_dram_parameter("wqkv", [1024, 3072], bf16, isOutput=False)
    bias_d = nc.declare_dram_parameter("bqkv", [128, 3072], bf16, isOutput=False)
    wo_d = nc.declare_dram_parameter("wo", [1024, 1024], bf16, isOutput=False)
    bo_d = nc.declare_dram_parameter("bo", [1024, 1], f32, isOutput=False)
    out_d = nc.declare_dram_parameter("out", [1024, ROWS], f32, isOutput=True)

    from contextlib import ExitStack
    with ExitStack() as ctx:
        tc = ctx.enter_context(tile.TileContext(nc))
        const = ctx.enter_context(tc.tile_pool(name="const", bufs=1))
        work = ctx.enter_context(tc.tile_pool(name="work", bufs=2))
        psq = ctx.enter_context(tc.tile_pool(name="psq", bufs=2, space="PSUM"))
        pss = ctx.enter_context(tc.tile_pool(name="pss", bufs=1, space="PSUM"))
        drp = ctx.enter_context(tc.tile_pool(name="drp", bufs=2, space="DRAM"))

        # ---- persistent tensors ----
        wq_sb = const.tile([128, 8 * 3072], bf16, tag="wq")
        wo_sb = const.tile([128, 8 * 1024], bf16, tag="wo")
        bias_sb = const.tile([128, 3072], bf16, tag="bias")
        bo_sb = const.tile([128, 8], f32, tag="bo")
        avb2 = const.tile([128, TILES * 1024], bf16, tag="avb2")
        qblk = const.tile([128, 2048], bf16, tag="qblk")
        kblk = const.tile([128, 1024], bf16, tag="kblk")
        ablk = const.tile([32, 2048], bf16, tag="ablk")
        vthp = const.tile([32, 4096], bf16, tag="vthp")

        # weight loads: wqkv chunk ec -> cols [3072*ec : +3072]
        src = xt_d  # placeholder to appease linters
        nc.sync.dma_start(
            wq_sb[:].rearrange("p (c f) -> p c f", c=8),
            wqkv_d[:].rearrange("(c p) f -> p c f", c=8),
        )
        nc.sync.dma_start(
            wo_sb[:].rearrange("p (q f) -> p q f", q=8),
            wo_d[:].rearrange("(q p) f -> p q f", q=8),
        )
        nc.sync.dma_start(bias_sb[:], bias_d[:])
        nc.sync.dma_start(
            bo_sb[:],
            bo_d[:].rearrange("(c p) one -> p c one", c=8)[:, :, 0],
        )
        # structural zeros (written once; per-tile copies only touch diag blocks)
        nc.vector.memset(qblk[:], 0.0)
        nc.vector.memset(ablk[:], 0.0)

        for t in range(TILES):
            # ---- load xT tile: xt_sb[:, 128c:+128] = xt_d[128c:+128, 128t:+128]
            xt_sb = work.tile([128, 1024], bf16, tag="xt")
            nc.sync.dma_start(
                xt_sb[:].rearrange("p (c s) -> p c s", c=8),
                xt_d[:].rearrange("(c p) s -> p c s", c=8)[:, :, 128 * t:128 * (t + 1)],
            )
            # ---- projections (Q, K, V sequentially through 2-bank psum) ----
            qn = work.tile([128, 1024], bf16, tag="qn")
            kn = work.tile([128, 1024], bf16, tag="kn")
            vn = work.tile([128, 1024], bf16, tag="vn")
            for w, dst in enumerate((qn, kn, vn)):
                psp = psq.tile([128, 1024], f32, tag="psp", name="psp")
                for ec in range(8):
                    lhsT = xt_sb[:, 128 * ec:128 * (ec + 1)]
                    for half in range(2):
                        rhs = wq_sb[:, 3072 * ec + 1024 * w + 512 * half:
                                    3072 * ec + 1024 * w + 512 * (half + 1)]
                        nc.tensor.matmul(
                            psp[:, 512 * half:512 * (half + 1)], lhsT, rhs,
                            start=(ec == 0), stop=(ec == 7))
                nc.vector.tensor_add(dst[:], psp[:], bias_sb[:, 1024 * w:1024 * (w + 1)])
            # ---- xbar transposes Q,K ----
            qt = work.tile([128, 1024], bf16, tag="qt")
            kt = work.tile([128, 1024], bf16, tag="kt")
            for c in range(8):
                nc.sync.dma_start_transpose(qt[:, 128 * c:128 * (c + 1)], qn[:, 128 * c:128 * (c + 1)])
                nc.sync.dma_start_transpose(kt[:, 128 * c:128 * (c + 1)], kn[:, 128 * c:128 * (c + 1)])
            # ---- QBLK / KBLK fused copies ----
            for p in range(2):
                for i2 in range(2):
                    srcq = qt[:][64 * i2:64 * i2 + 64, :].rearrange(
                        "p (c s) -> p c s", c=8)[:, :, 64 * p:64 * p + 64]
                    dstq = qblk[:][64 * p:64 * p + 64, 1024 * p:1024 * p + 1024].rearrange(
                        "p (c a g) -> p c a g", c=8, a=2)[:, :, i2, :]
                    nc.vector.tensor_copy(dstq, srcq)
                    srck = kt[:][64 * i2:64 * i2 + 64, :].rearrange(
                        "p (c s) -> p c s", c=8)[:, :, 64 * p:64 * p + 64]
                    dstk = kblk[:][64 * p:64 * p + 64, :].rearrange(
                        "p (c a g) -> p c a g", c=8, a=2)[:, :, i2, :]
                    nc.vector.tensor_copy(dstk, srck)
            # ---- scores: 64 pair matmuls ----
            sco = pss.tile([128, 256], f32, tag="sco")
            qv = qblk[:].rearrange("p (a i g) -> p a i g", a=2, i=16)
            kv = kblk[:].rearrange("p (j g) -> p j g", j=16)
            scov = sco[:].rearrange("p (j gg) -> p j gg", j=16)
            for g in range(64):
                u, gg = g % 4, g // 4
                nc.tensor.matmul(
                    scov[32 * u:32 * u + 32, :, gg],
                    qv[:, :, :, g], kv[:, :, g],
                    start=True, stop=True, tile_position=(0, 32 * u))
            # ---- softmax pieces ----
            ex = work.tile([128, 256], bf16, tag="ex")
            nc.scalar.activation(ex[:], sco[:], func=AF.Exp)
            z = work.tile([128, 16], f32, tag="z")
            zr = work.tile([128, 16], f32, tag="zr")
            nc.vector.tensor_reduce(
                z[:], ex[:].rearrange("p (j gg) -> p gg j", j=16),
                axis=AX.X, op=ALU.add)
            nc.vector.reciprocal(zr[:], z[:])
            # ---- bounce E -> ABLK ----
            exd = drp.tile([128, 256], bf16, tag="exd")
            nc.sync.dma_start(exd[:], ex[:])
            for p in range(2):
                for u in range(4):
                    dst = ablk[:][16 * p:16 * p + 16, 1024 * p:1024 * p + 1024].rearrange(
                        "P (i u gg) -> P i u gg", i=16, u=4)[:, :, u, :]
                    srce = exd[:].rearrange(
                        "(u a i) (j gg) -> u a j i gg", u=4, a=2, i=16, j=16)[u, p]
                    nc.sync.dma_start(dst, srce)
            # ---- bounce VN -> VTHP ----
            vnd = drp.tile([128, 1024], bf16, tag="vnd")
            nc.sync.dma_start(vnd[:], vn[:])
            for p in range(2):
                nc.sync.dma_start(
                    vthp[:][16 * p:16 * p + 16, :].rearrange("P (g d) -> P g d", g=64),
                    vnd[:].rearrange("(a g) (j d) -> a j g d", a=2, j=16)[p])
            # ---- AV: 64 pair matmuls ----
            anat = pss.tile([128, 1024], f32, tag="anat")
            av = ablk[:].rearrange("P (a i u gg) -> P a i u gg", a=2, i=16, u=4)
            vv = vthp[:].rearrange("P (g d) -> P g d", g=64)
            anv = anat[:].rearrange("p (gg d) -> p gg d", gg=16)
            for g in range(64):
                u, gg = g % 4, g // 4
                nc.tensor.matmul(
                    anv[32 * u:32 * u + 32, gg, :],
                    av[:, :, :, u, gg], vv[:, g, :],
                    start=True, stop=True, tile_position=(0, 32 * u))
            # ---- normalize by 1/Z and evac ----
            anat_sb = work.tile([128, 1024], bf16, tag="anat_sb")
            asv = anat_sb[:].rearrange("p (gg d) -> p gg d", gg=16)
            for gg in range(16):
                nc.vector.tensor_scalar_mul(asv[:, gg, :], anv[:, gg, :], zr[:, gg:gg + 1])
            # ---- xbar ANAT -> AVB2 ----
            for c4 in range(8):
                nc.sync.dma_start_transpose(
                    avb2[:, 1024 * t + 128 * c4:1024 * t + 128 * (c4 + 1)],
                    anat_sb[:, 128 * c4:128 * (c4 + 1)])

        # ---- final projection ----
        avv = avb2[:].rearrange("p (t c u a i) -> p t c u a i", t=TILES, c=8, u=4, a=2)
        for fc in range(8):
            for tg in range(4):
                psf = pss.tile([128, 512], f32, tag="psf")
                for q in range(8):
                    u, pq = q // 2, q % 2
                    nc.tensor.matmul(
                        psf[:], wo_sb[:, 1024 * q + 128 * fc:1024 * q + 128 * (fc + 1)],
                        avv[:, 4 * tg:4 * (tg + 1), :, u, pq, :],
                        start=(q == 0), stop=(q == 7))
                osb = work.tile([128, 512], f32, tag="osb")
                nc.vector.tensor_scalar_add(osb[:], psf[:], bo_sb[:, fc:fc + 1])
                nc.sync.dma_start(out_d[128 * fc:128 * (fc + 1), 512 * tg:512 * (tg + 1)], osb[:])
    return nc


def _host_prep(x, Wq, bq, Wk, bk, Wv, bv, Wo, bo):
    """Returns per-core input maps + post-scatter metadata."""
    xf = np.ascontiguousarray(x.reshape(NB * SB, EB))
    # slot permutation within each 128-tile: col 128t + 64p + g <- row 128t+2g+p
    idx = np.arange(ROWS)
    t, r = idx // 128, idx % 128
    p, g = r // 64, r % 64
    perm = 128 * t + 2 * g + p
    Wqs, bqs = Wq / 8.0, bq / 8.0
    WQKVT = np.concatenate([Wqs.T, Wk.T, Wv.T], axis=1).astype(ml_dtypes.bfloat16)
    BQKV = np.tile(np.concatenate([bqs, bk, bv])[None, :], (128, 1)).astype(ml_dtypes.bfloat16)
    WoTP = np.zeros((1024, 1024), np.float32)
    for u in range(4):
        for pp in range(2):
            q = 2 * u + pp
            for b in range(2):
                m = 8 * b + 2 * u + pp
                WoTP[q * 128 + b * 64:q * 128 + (b + 1) * 64, :] = Wo[:, m * 64:(m + 1) * 64].T
    WoTP = WoTP.astype(ml_dtypes.bfloat16)
    boT = bo.reshape(1024, 1).astype(np.float32)
    in_maps = []
    for core in range(8):
        n, s0 = core // 2, (core % 2) * ROWS
        xs = xf[n * SB + s0:n * SB + s0 + ROWS][perm]
        xT = np.ascontiguousarray(xs.T).astype(ml_dtypes.bfloat16)
        in_maps.append({"xt": xT, "wqkv": WQKVT, "bqkv": BQKV, "wo": WoTP, "bo": boT})
    return in_maps


def kernel(x, Wq, bq, Wk, bk, Wv, bv, Wo, bo):
    _install_birpatch()
    from concourse.bass_utils import run_bass_kernel_spmd

    if "nc" not in _CACHE:
        _CACHE["nc"] = _build_bass()
    nc = _CACHE["nc"]
    in_maps = _host_prep(np.asarray(x, np.float32), *[np.asarray(a, np.float32)
                         for a in (Wq, bq, Wk, bk, Wv, bv, Wo, bo)])
    res = run_bass_kernel_spmd(nc, in_maps, list(range(8)))
    out = np.zeros((NB, SB, EB), np.float32)
    # col t*128 + c4*16 + h -> row s_out = h*256 + (s0/16 + 8t + c4)
    tt = np.arange(ROWS)
    ct, cc4, ch = tt // 128, (tt // 16) % 8, tt % 16
    for core in range(8):
        n, s0 = core // 2, (core % 2) * ROWS
        fT = np.asarray(res.results[core]["out"])  # (1024, 2048)
        rows = ch * 256 + (s0 // 16 + 8 * ct + cc4)
        out[n, rows, :] = fT.T
    return out
